# revision 18
# baseline (speedup 1.0000x reference)
"""Trainium2 Bass kernel for nn_DilateResNetBidirection.

Sharding: 8 cores = 4 samples x 2 directions (pure data parallel).
Per core: full backbone in fp32 shifted-matmul convs (K=128 tap pairs,
dual-stream PE col-tiling), fused convf+softmax, m_kernel convs; a tiny
second launch combines the two directions (attn blend).
"""
import sys
import numpy as np

sys.path.insert(0, '/opt/trn_rl_repo')

import concourse.bacc as bacc
import concourse.bass as bass
import concourse.mybir as mybir
import concourse.tile as tile
import concourse.bass_isa as ISA

F32 = mybir.dt.float32
ds = bass.ds
AF = mybir.ActivationFunctionType
OP = mybir.AluOpType

NB = 10          # residual blocks
R = 8            # output rows per band iteration

# conv1 (w11 pad1 + w12 dil2 pad2 fused, 18 taps) pass table:
# each entry: (kernel, kyA, kxA, kernel2, kyB, kxB, dx) with tapB = tapA+(2,0)
# kernel index: 0=w11, 1=w12; ky/kx are 0..2 kernel indices; dx = col shift
C1_PASSES = [
    ('w11', 0, 0, 'w11', 2, 0, -1),   # (-1,-1)+(1,-1)
    ('w11', 0, 1, 'w11', 2, 1, 0),    # (-1,0)+(1,0)
    ('w11', 0, 2, 'w11', 2, 2, 1),    # (-1,1)+(1,1)
    ('w12', 0, 0, 'w12', 1, 0, -2),   # (-2,-2)+(0,-2)
    ('w12', 0, 1, 'w12', 1, 1, 0),    # (-2,0)+(0,0)
    ('w12', 0, 2, 'w12', 1, 2, 2),    # (-2,2)+(0,2)
    ('w11', 1, 1, 'w12', 2, 1, 0),    # w11(0,0)+w12(2,0)
    ('w11', 1, 0, None, 0, 0, -1),    # w11(0,-1) single
    ('w11', 1, 2, None, 0, 0, 1),     # w11(0,1) single
    ('w12', 2, 0, None, 0, 0, -2),    # w12(2,-2) single
    ('w12', 2, 2, None, 0, 0, 2),     # w12(2,2) single
]
# base tap dy for each conv1 pass (group1 read row offset)
C1_DY = [-1, -1, -1, -2, -2, -2, 0, 0, 0, 2, 2]

# conv2 / convf (3x3 pad 1) pass table: (kyA, kxA, kyB, kxB, dx), dyA=-1 pairs
C2_PASSES = [
    (0, 0, 2, 0, -1),   # (-1,-1)+(1,-1)
    (0, 1, 2, 1, 0),
    (0, 2, 2, 2, 1),
    (1, 0, None, None, -1),  # (0,-1) single
    (1, 1, None, None, 0),
    (1, 2, None, None, 1),
]
C2_DY = [-1, -1, -1, 0, 0, 0]


def pack_host_inputs(inp):
    """Pure data-movement packing of the problem inputs (shared across cores)."""
    out = {}
    cw = np.ascontiguousarray
    # conv0 weights: [54, 64], row = 6*(3*ky+kx) + c
    out['w0'] = cw(np.transpose(inp['conv0_w'], (2, 3, 1, 0)).reshape(54, 64))

    # block pass weights: [NB*17, 128, 64]
    wall = np.zeros((NB * 17, 128, 64), np.float32)
    for b in range(NB):
        ws = {'w11': inp['blk_w11'][b], 'w12': inp['blk_w12'][b]}
        for p, (ka, kya, kxa, kb, kyb, kxb, dx) in enumerate(C1_PASSES):
            wall[b * 17 + p, 0:64, :] = np.transpose(ws[ka][:, :, kya, kxa])
            if kb is not None:
                wall[b * 17 + p, 64:128, :] = np.transpose(ws[kb][:, :, kyb, kxb])
        w2 = inp['blk_w2'][b]
        for p, (kya, kxa, kyb, kxb, dx) in enumerate(C2_PASSES):
            wall[b * 17 + 11 + p, 0:64, :] = np.transpose(w2[:, :, kya, kxa])
            if kyb is not None:
                wall[b * 17 + 11 + p, 64:128, :] = np.transpose(w2[:, :, kyb, kxb])
    out['wall'] = wall

    wf = np.zeros((6, 128, 25), np.float32)
    for p, (kya, kxa, kyb, kxb, dx) in enumerate(C2_PASSES):
        wf[p, 0:64, :] = np.transpose(inp['convf_w'][:, :, kya, kxa])
        if kyb is not None:
            wf[p, 64:128, :] = np.transpose(inp['convf_w'][:, :, kyb, kxb])
    out['wf'] = wf

    # m_kernel: [5 dy][125, 1]; row 25*g + cc = m[cc, dy, dx=g-2]
    m5 = np.zeros((5, 125, 1), np.float32)
    mk = inp['m_kernel'][0]  # [25, 5, 5]
    for dy in range(5):
        for g in range(5):
            m5[dy, 25 * g:25 * g + 25, 0] = mk[:, dy, g]
    out['m5'] = m5

    for nm in ('g', 'b', 'm', 'v'):
        out['bn1' + nm] = cw(np.transpose(inp['blk_bn1_' + nm]))  # [64, NB]
        out['bn2' + nm] = cw(np.transpose(inp['blk_bn2_' + nm]))
        out['bnf' + nm] = cw(inp['bnf_' + nm].reshape(64, 1))
        out['bn0' + nm] = cw(inp['bn0_' + nm].reshape(6, 1))
    out['cfb'] = cw(inp['convf_b'].reshape(25, 1))
    return out


def build_main(H=256, nb=NB):
    """Launch-1 program: backbone + softmax + m_kernel convs for one
    (sample, direction) unit per core."""
    HH = H // 2                 # rows per stream
    NIT = HH // R               # band iterations
    assert HH % R == 0

    nc = bacc.Bacc("TRN2", target_bir_lowering=False, debug=False)

    # ---- dram I/O
    xpad_d = nc.dram_tensor("xpad", [6, H + 4, 260], F32, kind="ExternalInput")
    iml_d = nc.dram_tensor("imlast", [3, H, 256], F32, kind="ExternalInput")
    w0_d = nc.dram_tensor("w0", [54, 64], F32, kind="ExternalInput")
    wall_d = nc.dram_tensor("wall", [NB * 17, 128, 64], F32, kind="ExternalInput")
    wf_d = nc.dram_tensor("wf", [6, 128, 25], F32, kind="ExternalInput")
    m5_d = nc.dram_tensor("m5", [5, 125, 1], F32, kind="ExternalInput")
    bnt = {}
    for nm in ('g', 'b', 'm', 'v'):
        bnt['1' + nm] = nc.dram_tensor("bn1" + nm, [64, NB], F32, kind="ExternalInput")
        bnt['2' + nm] = nc.dram_tensor("bn2" + nm, [64, NB], F32, kind="ExternalInput")
        bnt['f' + nm] = nc.dram_tensor("bnf" + nm, [64, 1], F32, kind="ExternalInput")
        bnt['0' + nm] = nc.dram_tensor("bn0" + nm, [6, 1], F32, kind="ExternalInput")
    cfb_d = nc.dram_tensor("cfb", [25, 1], F32, kind="ExternalInput")

    mask_d = nc.dram_tensor("mask", [25, H, 256], F32, kind="ExternalOutput")
    segpred_d = nc.dram_tensor("segpred", [4, H, 256], F32, kind="ExternalOutput")

    with tile.TileContext(nc) as tc:
      with (
        tc.tile_pool(name="dram", bufs=1, space="DRAM") as dpool,
        tc.tile_pool(name="stat", bufs=1) as st,
        tc.tile_pool(name="pre_ps", bufs=1, space="PSUM") as pps,
      ):
        hA = dpool.tile([64, H + 8, 260], F32)
        hB = dpool.tile([64, H + 8, 260], F32)
        X_dram = dpool.tile([100, H + 4, 260], F32)
        nc.dbg_names = {'hA': hA, 'hB': hB, 'X': X_dram}

        # ---- static param tiles
        w0s = st.tile([54, 64], F32)
        nc.sync.dma_start(w0s[:], w0_d[:])
        wfs = st.tile([128, 6, 25], F32)
        nc.sync.dma_start(wfs[:], wf_d[:].rearrange("p k m -> k p m"))
        m5s = st.tile([125, 5], F32)
        nc.sync.dma_start(m5s[:], m5_d[:].rearrange("p k m -> k (p m)"))
        bnsb = {}
        for key, d in bnt.items():
            if key.startswith('0'):
                t = st.tile([6, 1], F32, tag="bn" + key)
                nc.sync.dma_start(t[:], d[:])
            else:
                t = st.tile([128, d.shape[1]], F32, tag="bn" + key)
                nc.sync.dma_start(t[0:64, :], d[:])
                nc.sync.dma_start(t[64:128, :], d[:])
            bnsb[key] = t
        cfbs = st.tile([128, 1], F32)
        nc.sync.dma_start(cfbs[0:25, :], cfb_d[:])
        nc.sync.dma_start(cfbs[64:89, :], cfb_d[:])

        # ---- bn folds: inv = g/sqrt(v+eps), beta = b - m*inv,
        #      padv = -(beta+1)*recip(inv)  (relu pad killer)
        def bn_fold(pfx, P, W, relu_pad):
            g, b, m, v = (bnsb[pfx + nm] for nm in ('g', 'b', 'm', 'v'))
            eps = st.tile([P, 1], F32, tag="f_eps" + pfx)
            nc.vector.memset(eps[:], 1e-5)
            sq = st.tile([P, W], F32, tag="f_sq" + pfx)
            nc.scalar.activation(sq[:], v[:], AF.Sqrt, bias=eps[:, 0:1], scale=1.0)
            rs = st.tile([P, W], F32, tag="f_rs" + pfx)
            nc.vector.reciprocal(rs[:], sq[:])
            inv = st.tile([P, W], F32, tag="f_inv" + pfx)
            nc.vector.tensor_mul(inv[:], g[:], rs[:])
            mi = st.tile([P, W], F32, tag="f_mi" + pfx)
            nc.vector.tensor_mul(mi[:], m[:], inv[:])
            beta = st.tile([P, W], F32, tag="f_beta" + pfx)
            nc.vector.tensor_sub(beta[:], b[:], mi[:])
            rinv = st.tile([P, W], F32, tag="f_rinv" + pfx)
            nc.vector.reciprocal(rinv[:], inv[:])
            bp = st.tile([P, W], F32, tag="f_bp" + pfx)
            if relu_pad:
                nc.vector.tensor_scalar(bp[:], beta[:], 1.0, None, OP.add)
            else:
                nc.vector.tensor_copy(bp[:], beta[:])
            pv = st.tile([P, W], F32, tag="f_pv" + pfx)
            nc.vector.tensor_mul(pv[:], bp[:], rinv[:])
            nc.vector.tensor_scalar(pv[:], pv[:], -1.0, None, OP.mult)
            return inv, beta, pv

        inv1, bet1, pv1 = bn_fold('1', 128, NB, True)
        inv2, bet2, _ = bn_fold('2', 128, NB, True)
        invf, betf, pvf = bn_fold('f', 128, 1, True)
        inv0, bet0, pv0 = bn_fold('0', 6, 1, False)

        # replicate inv0/bet0 to 54 rows (9 tap groups) via DRAM bounce
        bn0scr = dpool.tile([2, 6, 1], F32)
        nc.sync.dma_start(bn0scr[0], inv0[:])
        nc.sync.dma_start(bn0scr[1], bet0[:])
        inv0r = st.tile([54, 1], F32)
        bet0r = st.tile([54, 1], F32)
        for g in range(9):
            nc.sync.dma_start(inv0r[6 * g:6 * g + 6, :], bn0scr[0])
            nc.sync.dma_start(bet0r[6 * g:6 * g + 6, :], bn0scr[1])
        # fold bn0 into conv0 weights + bias
        w0p = st.tile([54, 64], F32)
        nc.vector.tensor_scalar(w0p[:], w0s[:], inv0r[:, 0:1], None, OP.mult)
        b0ps = pps.tile([64, 1], F32)
        nc.tensor.matmul(b0ps[:], w0s[:], bet0r[:], start=True, stop=True)
        b0t = st.tile([64, 1], F32)
        nc.vector.tensor_copy(b0t[:], b0ps[:])
        b0scr = dpool.tile([64, 1], F32)
        nc.sync.dma_start(b0scr[:], b0t[:])
        bias0 = st.tile([128, 1], F32)
        nc.sync.dma_start(bias0[0:64, :], b0scr[:])
        nc.sync.dma_start(bias0[64:128, :], b0scr[:])

        ones25 = st.tile([25, 1], F32)
        nc.vector.memset(ones25[:], 1.0)

        # ---- x_pad pad strips <- padv0 (per-channel)
        pvrow = st.tile([6, 2, 260], F32)
        nc.vector.memset(pvrow[:], 0.0)
        nc.vector.tensor_scalar(pvrow[:], pvrow[:], pv0[:, 0:1], None, OP.add)
        pvcol = st.tile([6, 256, 2], F32)
        nc.vector.memset(pvcol[:], 0.0)
        nc.vector.tensor_scalar(pvcol[:], pvcol[:], pv0[:, 0:1], None, OP.add)
        nc.sync.dma_start(xpad_d[:, 0:2, :], pvrow[:])
        nc.sync.dma_start(xpad_d[:, H + 2:H + 4, :], pvrow[:])
        nc.sync.dma_start(xpad_d[:, 2:H + 2, 0:2], pvcol[:, 0:H, :])
        nc.sync.dma_start(xpad_d[:, 2:H + 2, 258:260], pvcol[:, 0:H, :])

        # ---- X_dram zero pad strips
        zrow = st.tile([100, 2, 260], F32)
        nc.vector.memset(zrow[:], 0.0)
        zcol = st.tile([100, 256, 2], F32)
        nc.vector.memset(zcol[:], 0.0)
        nc.sync.dma_start(X_dram[:, 0:2, :], zrow[:])
        nc.sync.dma_start(X_dram[:, H + 2:H + 4, :], zrow[:])
        nc.sync.dma_start(X_dram[:, 2:H + 2, 0:2], zcol[:, 0:H, :])
        nc.sync.dma_start(X_dram[:, 2:H + 2, 258:260], zcol[:, 0:H, :])
        for hbuf in (hA, hB):
            nc.sync.dma_start(hbuf[:, 4:H + 4, 0:2], zcol[0:64, 0:H, :])
            nc.sync.dma_start(hbuf[:, 4:H + 4, 258:260], zcol[0:64, 0:H, :])

        # h pad-row writer (per-block pad values)
        def write_h_pads(hbuf, pvap):
            pr = st.tile([64, 4, 260], F32, tag="hpadrow")
            nc.vector.memset(pr[:], 0.0)
            nc.vector.tensor_scalar(pr[:], pr[:], pvap, None, OP.add)
            nc.sync.dma_start(hbuf[:, 0:4, :], pr[:])
            nc.sync.dma_start(hbuf[:, H + 4:H + 8, :], pr[:])

        # =================== P1: conv0 ===================
        with (
            tc.tile_pool(name="c0_x", bufs=3) as c0x,
            tc.tile_pool(name="c0_o", bufs=3) as c0o,
            tc.tile_pool(name="c0_ps", bufs=2, space="PSUM") as c0ps,
        ):
            with tc.For_i(0, HH // 2) as i:
                x54a = c0x.tile([54, 2, 256], F32, tag="x54a")
                x54b = c0x.tile([54, 2, 256], F32, tag="x54b")
                for gy in range(3):
                    for gx in range(3):
                        g = 3 * gy + gx
                        nc.sync.dma_start(
                            x54a[6 * g:6 * g + 6, :, :],
                            xpad_d[:, ds(i * 2 + gy + 1, 2), gx + 1:gx + 257])
                        nc.sync.dma_start(
                            x54b[6 * g:6 * g + 6, :, :],
                            xpad_d[:, ds(i * 2 + HH + gy + 1, 2), gx + 1:gx + 257])
                pc = c0ps.tile([128, 2, 256], F32, tag="c0ps")
                nc.tensor.matmul(pc[0:64, :, :], w0p[:], x54a[:],
                                 start=True, stop=True,
                                 tile_position=(0, 0), skip_group_check=True)
                nc.tensor.matmul(pc[64:128, :, :], w0p[:], x54b[:],
                                 start=True, stop=True,
                                 tile_position=(0, 64), skip_group_check=True)
                h0 = c0o.tile([128, 2, 256], F32, tag="h0")
                nc.scalar.activation(h0[0:64, :, :], pc[0:64, :, :], AF.Identity,
                                     bias=bias0[0:64, 0:1], scale=1.0)
                nc.scalar.activation(h0[64:128, :, :], pc[64:128, :, :], AF.Identity,
                                     bias=bias0[64:128, 0:1], scale=1.0)
                nc.sync.dma_start(hA[:, ds(i * 2 + 4, 2), 2:258], h0[0:64, :, :])
                nc.sync.dma_start(hA[:, ds(i * 2 + HH + 4, 2), 2:258], h0[64:128, :, :])

        # =================== P2: residual blocks ===================
        NCH = (R + 2) // 2      # t chunks per iter (5)
        with (
            tc.tile_pool(name="bk_w", bufs=2) as bkw,
            tc.tile_pool(name="bk_h", bufs=2) as bkh,
            tc.tile_pool(name="bk_a", bufs=2) as bka,
            tc.tile_pool(name="bk_t", bufs=2) as bkt,
            tc.tile_pool(name="bk_o", bufs=3) as bko,
            tc.tile_pool(name="bk_ps1", bufs=1, space="PSUM") as bps1,
            tc.tile_pool(name="bk_ps2", bufs=2, space="PSUM") as bps2,
        ):
            for blk in range(nb):
                hin_d, hout_d = (hA, hB) if blk % 2 == 0 else (hB, hA)
                write_h_pads(hin_d, pv1[0:64, blk:blk + 1])
                wblk = bkw.tile([128, 17, 64], F32, tag="wblk")
                nc.sync.dma_start(
                    wblk[:], wall_d[17 * blk:17 * blk + 17].rearrange("p k m -> k p m"))

                with tc.For_i(0, NIT) as i:
                    hinT = bkh.tile([128, 14, 260], F32, tag="hin")
                    nc.sync.dma_start(hinT[0:64, :, :], hin_d[:, ds(i * R + 1, 14), :])
                    nc.sync.dma_start(hinT[64:128, :, :],
                                      hin_d[:, ds(i * R + HH + 1, 14), :])
                    for s in range(2):          # streams
                        sl = slice(64 * s, 64 * s + 64)
                        tpos = (0, 64 * s)
                        hin = hinT[sl, :, :]
                        A = bka.tile([128, 14, 260], F32, tag=f"A{s}")
                        nc.vector.memset(A[:, :, 0:2], 0.0)
                        nc.vector.memset(A[:, :, 258:260], 0.0)
                        nc.vector.memset(A[64:128, 12:14, :], 0.0)
                        bsl = slice(64 * s, 64 * s + 64)
                        nc.scalar.activation(A[0:64, 0:14, 2:258], hin[:, 0:14, 2:258],
                                             AF.Relu, bias=bet1[bsl, blk:blk + 1],
                                             scale=inv1[bsl, blk:blk + 1])
                        nc.scalar.activation(A[64:128, 0:12, 2:258], hin[:, 2:14, 2:258],
                                             AF.Relu, bias=bet1[bsl, blk:blk + 1],
                                             scale=inv1[bsl, blk:blk + 1])
                        traw = bps1.tile([128, R + 2, 256], F32, tag="traw")
                        for c in range(NCH):
                            for p in range(11):
                                dy = C1_DY[p]
                                dx = C1_PASSES[p][6]
                                nc.tensor.matmul(
                                    traw[sl, 2 * c:2 * c + 2, :],
                                    wblk[:, p, :],
                                    A[:, 2 * c + 2 + dy:2 * c + 4 + dy,
                                      2 + dx:258 + dx],
                                    start=(p == 0), stop=(p == 10),
                                    tile_position=tpos, skip_group_check=True)
                        T = bkt.tile([128, R + 2, 260], F32, tag=f"T{s}")
                        nc.vector.memset(T[:, :, 0:2], 0.0)
                        nc.vector.memset(T[:, :, 258:260], 0.0)
                        nc.vector.memset(T[64:128, R:R + 2, :], 0.0)
                        nc.scalar.activation(T[0:64, 0:R + 2, 2:258],
                                             traw[sl, 0:R + 2, :], AF.Relu,
                                             bias=bet2[bsl, blk:blk + 1],
                                             scale=inv2[bsl, blk:blk + 1])
                        nc.scalar.activation(T[64:128, 0:R, 2:258],
                                             traw[sl, 2:R + 2, :], AF.Relu,
                                             bias=bet2[bsl, blk:blk + 1],
                                             scale=inv2[bsl, blk:blk + 1])
                        if s == 0:
                            with tc.If(i < 1):
                                nc.vector.memset(T[0:64, 0:1, 2:258], 0.0)
                        else:
                            with tc.If(i > NIT - 2):
                                nc.vector.memset(T[64:128, R - 1:R, 2:258], 0.0)
                        for c in range(R // 2):
                            p2 = bps2.tile([128, 2, 256], F32, tag="p2")
                            for p in range(6):
                                dy = C2_DY[p]
                                dx = C2_PASSES[p][4]
                                nc.tensor.matmul(
                                    p2[sl, :, :],
                                    wblk[:, 11 + p, :],
                                    T[:, 2 * c + 1 + dy:2 * c + 3 + dy,
                                      2 + dx:258 + dx],
                                    start=(p == 0), stop=(p == 5),
                                    tile_position=tpos, skip_group_check=True)
                            ho = bko.tile([64, 2, 256], F32, tag="ho")
                            nc.vector.tensor_tensor(
                                ho[:], p2[sl, :, :],
                                hin[:, 2 * c + 3:2 * c + 5, 2:258], OP.add)
                            nc.sync.dma_start(
                                hout_d[:, ds(i * R + HH * s + 2 * c + 4, 2), 2:258],
                                ho[:])

        # =================== P3: convf + softmax + X planes ===================
        hfin = hA if nb % 2 == 0 else hB
        write_h_pads(hfin, pvf[0:64, 0:1])
        with (
            tc.tile_pool(name="cf_h", bufs=2) as cfh,
            tc.tile_pool(name="cf_a", bufs=2) as cfa,
            tc.tile_pool(name="cf_w", bufs=1) as cfw,
            tc.tile_pool(name="cf_ps", bufs=1, space="PSUM") as cfps,
            tc.tile_pool(name="cf_pss", bufs=3, space="PSUM") as cfpss,
        ):
            with tc.For_i(0, NIT) as i:
                hfT = cfh.tile([128, R + 3, 260], F32, tag="hf")
                nc.sync.dma_start(hfT[0:64, :, :], hfin[:, ds(i * R + 3, R + 3), :])
                nc.sync.dma_start(hfT[64:128, :, :], hfin[:, ds(i * R + HH + 3, R + 3), :])
                imbA = cfh.tile([128, R, 256], F32, tag="imbA")
                imbB = cfh.tile([128, R, 256], F32, tag="imbB")
                for ch in range(3):
                    nc.sync.dma_start(imbA[32 * ch:32 * ch + 1, :, :],
                                      iml_d[ch, ds(i * R, R), :])
                    nc.sync.dma_start(imbB[32 * ch:32 * ch + 1, :, :],
                                      iml_d[ch, ds(i * R + HH, R), :])
                pF = cfps.tile([128, R, 256], F32, tag="pF")
                for s in range(2):
                    sl64 = slice(64 * s, 64 * s + 64)
                    Af = cfa.tile([128, R + 1, 260], F32, tag=f"Af{s}")
                    nc.vector.memset(Af[:, :, 0:2], 0.0)
                    nc.vector.memset(Af[:, :, 258:260], 0.0)
                    nc.scalar.activation(Af[0:64, 0:R + 1, 2:258],
                                         hfT[sl64, 0:R + 1, 2:258], AF.Relu,
                                         bias=betf[sl64, 0:1], scale=invf[sl64, 0:1])
                    nc.scalar.activation(Af[64:128, 0:R + 1, 2:258],
                                         hfT[sl64, 2:R + 3, 2:258], AF.Relu,
                                         bias=betf[sl64, 0:1], scale=invf[sl64, 0:1])
                    for c in range(R // 2):
                        for p in range(6):
                            dy = C2_DY[p]
                            dx = C2_PASSES[p][4]
                            nc.tensor.matmul(
                                pF[64 * s:64 * s + 25, 2 * c:2 * c + 2, :],
                                wfs[:, p, :],
                                Af[:, 2 * c + 1 + dy:2 * c + 3 + dy, 2 + dx:258 + dx],
                                start=(p == 0), stop=(p == 5),
                                tile_position=(0, 64 * s), skip_group_check=True)
                for s in range(2):
                    b = 64 * s
                    lsb = cfw.tile([128, R, 256], F32, tag="lsb")
                    nc.vector.tensor_copy(lsb[b:b + 25, :, :], pF[b:b + 25, :, :])
                    mxt = cfw.tile([128, R, 256], F32, tag="mxt")
                    nc.gpsimd.partition_all_reduce(
                        mxt[b:b + 25, :, :], lsb[b:b + 25, :, :], 25, ISA.ReduceOp.max)
                    dlg = cfw.tile([128, R, 256], F32, tag="dlg")
                    nc.vector.scalar_tensor_tensor(
                        dlg[b:b + 25, :, :], lsb[b:b + 25, :, :], cfbs[b:b + 25, 0:1],
                        mxt[b:b + 25, :, :], OP.add, OP.subtract)
                    esb = cfw.tile([128, R, 256], F32, tag="esb")
                    nc.scalar.activation(esb[b:b + 25, :, :], dlg[b:b + 25, :, :], AF.Exp)
                    ssb = cfw.tile([128, R, 256], F32, tag="ssb")
                    nc.gpsimd.partition_all_reduce(
                        ssb[b:b + 25, :, :], esb[b:b + 25, :, :], 25, ISA.ReduceOp.add)
                    rsb = cfw.tile([128, R, 256], F32, tag="rsb")
                    nc.vector.reciprocal(rsb[b:b + 25, :, :], ssb[b:b + 25, :, :])
                    msk = cfw.tile([128, R, 256], F32, tag="msk")
                    nc.vector.tensor_tensor(
                        msk[b:b + 25, :, :], esb[b:b + 25, :, :],
                        rsb[b:b + 25, :, :], OP.mult)
                    nc.sync.dma_start(mask_d[:, ds(i * R + HH * s, R), :],
                                      msk[b:b + 25, :, :])
                    nc.sync.dma_start(X_dram[0:25, ds(i * R + HH * s + 2, R), 2:258],
                                      msk[b:b + 25, :, :])
                    for ch in range(3):
                        imb = imbA if s == 0 else imbB
                        imbb = cfw.tile([128, R, 256], F32, tag="imbb")
                        nc.gpsimd.partition_broadcast(imbb[b:b + 25, :, :],
                                                      imb[32 * ch:32 * ch + 1, :, :])
                        prd = cfw.tile([128, R, 256], F32, tag=f"prd{ch}")
                        nc.vector.tensor_tensor(
                            prd[b:b + 25, :, :], msk[b:b + 25, :, :],
                            imbb[b:b + 25, :, :], OP.mult)
                        nc.sync.dma_start(
                            X_dram[25 * (ch + 1):25 * (ch + 2),
                                   ds(i * R + HH * s + 2, R), 2:258],
                            prd[b:b + 25, :, :])

        # =================== P4: m_kernel convs (seg + pred) ===================
        with (
            tc.tile_pool(name="mk_x", bufs=2) as mkx,
            tc.tile_pool(name="mk_o", bufs=3) as mko,
            tc.tile_pool(name="mk_ps", bufs=5, space="PSUM") as mkps,
        ):
            with tc.For_i(0, H // R) as i:
                pch = []
                for c in range(R // 2):
                    pcht = mkps.tile([128, 2, 256], F32, tag="mkps", name=f"mkps{c}")
                    pch.append(pcht)
                for conv in range(4):
                    x5 = mkx.tile([125, R + 4, 256], F32, tag="x5")
                    for g in range(5):
                        nc.sync.dma_start(
                            x5[25 * g:25 * g + 25, :, :],
                            X_dram[25 * conv:25 * conv + 25, ds(i * R, R + 4),
                                   g:g + 256])
                    for c in range(R // 2):
                        for dy in range(5):
                            nc.tensor.matmul(
                                pch[c][32 * conv:32 * conv + 1, :, :],
                                m5s[:, dy:dy + 1],
                                x5[:, 2 * c + dy:2 * c + dy + 2, :],
                                start=(dy == 0), stop=(dy == 4),
                                tile_position=(0, 32 * conv),
                                skip_group_check=True)
                for c in range(R // 2):
                    sg = mko.tile([128, 2, 256], F32, tag="sg")
                    for conv in range(4):
                        nc.vector.tensor_copy(sg[32 * conv:32 * conv + 1, :, :],
                                              pch[c][32 * conv:32 * conv + 1, :, :])
                    for conv in range(4):
                        nc.sync.dma_start(
                            segpred_d[conv, ds(i * R + 2 * c, 2), :],
                            sg[32 * conv:32 * conv + 1, :, :])

    nc.compile()
    return nc


def build_combine():
    """Launch-2: attn blend of the two directions (per-sample)."""
    nc = bacc.Bacc("TRN2", target_bir_lowering=False, debug=False)
    sf_d = nc.dram_tensor("sf", [128, 512], F32, kind="ExternalInput")
    sb_d = nc.dram_tensor("sb", [128, 512], F32, kind="ExternalInput")
    pf_d = nc.dram_tensor("pf", [3, 128, 512], F32, kind="ExternalInput")
    pb_d = nc.dram_tensor("pb", [3, 128, 512], F32, kind="ExternalInput")
    at_d = nc.dram_tensor("attn", [128, 512], F32, kind="ExternalOutput")
    ac_d = nc.dram_tensor("attnc", [128, 512], F32, kind="ExternalOutput")
    pr_d = nc.dram_tensor("pred", [3, 128, 512], F32, kind="ExternalOutput")
    with tile.TileContext(nc) as tc:
        with tc.tile_pool(name="sb_", bufs=1) as sp:
            sf = sp.tile([128, 512], F32)
            sbt = sp.tile([128, 512], F32)
            nc.sync.dma_start(sf[:], sf_d[:])
            nc.sync.dma_start(sbt[:], sb_d[:])
            den = sp.tile([128, 512], F32)
            nc.vector.tensor_tensor(den[:], sf[:], sbt[:], OP.add)
            nc.vector.tensor_scalar(den[:], den[:], 2e-5, None, OP.add)
            rec = sp.tile([128, 512], F32)
            nc.vector.reciprocal(rec[:], den[:])
            num = sp.tile([128, 512], F32)
            nc.vector.tensor_scalar(num[:], sf[:], 1e-5, None, OP.add)
            at = sp.tile([128, 512], F32)
            nc.vector.tensor_tensor(at[:], num[:], rec[:], OP.mult)
            ac = sp.tile([128, 512], F32)
            nc.vector.tensor_scalar(ac[:], at[:], -1.0, 1.0, OP.mult, OP.add)
            nc.sync.dma_start(at_d[:], at[:])
            nc.sync.dma_start(ac_d[:], ac[:])
            for c in range(3):
                pf = sp.tile([128, 512], F32, tag="pf")
                pb = sp.tile([128, 512], F32, tag="pb")
                nc.sync.dma_start(pf[:], pf_d[c])
                nc.sync.dma_start(pb[:], pb_d[c])
                t1 = sp.tile([128, 512], F32, tag="t1")
                nc.vector.tensor_tensor(t1[:], at[:], pf[:], OP.mult)
                t2 = sp.tile([128, 512], F32, tag="t2")
                nc.vector.tensor_tensor(t2[:], ac[:], pb[:], OP.mult)
                pr = sp.tile([128, 512], F32, tag="pr")
                nc.vector.tensor_tensor(pr[:], t1[:], t2[:], OP.add)
                nc.sync.dma_start(pr_d[c], pr[:])
    nc.compile()
    return nc


_NC_MAIN = None
_NC_COMB = None
TRACE = False
LAST_EXEC_NS = None


def kernel(**inputs):
    global _NC_MAIN, _NC_COMB
    inputs = {k: np.asarray(v, dtype=np.asarray(v).dtype) for k, v in inputs.items()}
    B, H = 4, 256
    shared = pack_host_inputs(inputs)

    if _NC_MAIN is None:
        _NC_MAIN = build_main(H)
    if _NC_COMB is None:
        _NC_COMB = build_combine()

    import trace_hook
    trace_hook.install()
    from concourse.bass_utils import run_bass_kernel_spmd

    in_maps = []
    for c in range(8):
        s, d = c // 2, c % 2
        im = inputs['im_input_f'] if d == 0 else inputs['im_input_b']
        xpad = np.zeros((6, H + 4, 260), np.float32)
        xpad[:, 2:H + 2, 2:258] = im[s]
        m = dict(shared)
        m['xpad'] = xpad
        m['imlast'] = np.ascontiguousarray(im[s, 3:6])
        in_maps.append(m)

    global LAST_EXEC_NS
    r1 = run_bass_kernel_spmd(_NC_MAIN, in_maps, list(range(8)), trace=TRACE)
    LAST_EXEC_NS = r1.exec_time_ns
    res1 = r1.results

    in2 = []
    for c in range(8):
        s = c // 2
        spf = res1[2 * s]['segpred']      # [4, 256, 256] forward
        spb = res1[2 * s + 1]['segpred']
        in2.append({
            'sf': spf[0].reshape(128, 512), 'sb': spb[0].reshape(128, 512),
            'pf': spf[1:4].reshape(3, 128, 512),
            'pb': spb[1:4].reshape(3, 128, 512),
        })
    res2 = run_bass_kernel_spmd(_NC_COMB, in2, list(range(8))).results

    mask_f = np.stack([res1[2 * s]['mask'] for s in range(B)])
    mask_b = np.stack([res1[2 * s + 1]['mask'] for s in range(B)])
    pred_f = np.stack([res1[2 * s]['segpred'][1:4] for s in range(B)])
    pred_b = np.stack([res1[2 * s + 1]['segpred'][1:4] for s in range(B)])
    attn = np.stack([res2[2 * s]['attn'].reshape(1, H, 256) for s in range(B)])
    attnc = np.stack([res2[2 * s]['attnc'].reshape(1, H, 256) for s in range(B)])
    pred = np.stack([res2[2 * s]['pred'].reshape(3, H, 256) for s in range(B)])
    return (pred, pred_f, mask_f, attn, pred_b, mask_b, attnc)
